# revision 1
# baseline (speedup 1.0000x reference)
"""DualPathSoftMoE2 Trainium2 kernel (8 NeuronCores, SPMD).

Key structural facts used (exact algebra, valid for ANY input values):
  - reference() replaces ALL occ-expert logits with -10000 before both the
    dispatch softmax and the combine entmax.  exp((-10000/s0)-max) underflows
    to exactly 0.0 in f32, so occ dispatch weights are exactly 0, occ slots
    are exactly 0, and the entmax support never reaches the occ entries
    (tau* >= -1 while occ z <= -5000), so occ combine weights are exactly 0.
    The occ path contributes exactly nothing to the output.
  - attn_weight is unused by reference().

Sharding: core c owns batch b=c for routing (phases A/C) and expert e=c for
the MLP (phase B).  Slots ([16,1024] per core) are exchanged with AllToAll.

Data-movement / scheduling plan (sim-guided; HBM traffic halved vs v1):
  - x, qt, w1, w2 are host-cast to bf16 (the matmuls consumed bf16 on-chip
    anyway); output is written bf16 and host-upcast.  32 MB HBM/core.
  - w1+w2 are prefetched in two 8 MB DMAs issued from gpsimd (Pool is idle
    until the first collective; SP streams x tiles, ACT does the PSUM->SBUF
    transpose copies), so phase B starts compute-bound, weights resident.
  - the 128x128 identity (PE-transpose operand) ships as a host input so
    nothing queues behind the weight DMAs.
  - raw logits accumulate into one persistent PSUM bank; softmax and the
    entmax z build read them in place (no per-tile PSUM->SBUF copy).
  - per-supertile softmax+slots are emitted half a supertile late so the
    in-order DVE never stalls tile ss ops behind a softmax.
  - both AllToAlls run in bf16; entmax Newton overlaps a2a1 + phase B;
    phase C writes alternate SP/Pool DMA rings, copies alternate DVE/ACT.

entmax-1.5 tau is found by Newton iteration on
f(tau) = sum(relu(z - tau)^2) - 1 from tau0 = -1 (left of the root, f convex
decreasing => monotone quadratic convergence; denominator >= 0.5 always since
tau* <= -0.25 for <=16 support entries).
"""

import sys

sys.path.insert(0, "/opt/trn_rl_repo")

import numpy as np
import ml_dtypes

import concourse.bass as bass
import concourse.mybir as mybir
import concourse.tile as tile
from concourse import bacc
from concourse.bass_utils import run_bass_kernel_spmd
from concourse.masks import make_identity

dt = mybir.dt
AF = mybir.ActivationFunctionType
ALU = mybir.AluOpType
AX = mybir.AxisListType

BF16 = ml_dtypes.bfloat16

# Problem shape (hardcoded per contract)
B, N, D = 8, 4096, 1024
NCEXP, S = 8, 2          # core experts / slots per expert
J = NCEXP * S            # 16 slot columns, e-major: j = 2e + s
HC = 4 * D               # core hidden
NT = N // 128            # 32 n-tiles per core
HT = HC // 128           # 32 h-tiles in the expert MLP
DC = D // 128            # 8 d-chunks
ST = 8                   # n-tiles per super-tile (softmax batch)
NST = NT // ST
L2_EPS = 1e-6
NEWTON_ITERS = 5
N_CORES = 8
RSQRT_MAGIC = 0x5F3759DF


def build_nc(n_repeat: int = 1, general_path: bool = False,
             n_rows: int = N, st_size: int = ST, debug: bool = False,
             taps: bool = False, stop_after: int = 99, sim_safe: bool = False):
    global N, NT, ST, NST
    N_sav, NT_sav, ST_sav, NST_sav = N, NT, ST, NST
    N, NT, ST, NST = n_rows, n_rows // 128, st_size, (n_rows // 128) // st_size
    try:
        return _build_nc_impl(n_repeat, general_path, debug, taps, stop_after,
                              sim_safe)
    finally:
        N, NT, ST, NST = N_sav, NT_sav, ST_sav, NST_sav


def _build_nc_impl(n_repeat: int, general_path: bool, debug: bool,
                   taps: bool = False, stop_after: int = 99,
                   sim_safe: bool = False):
    nc = bacc.Bacc("TRN2", target_bir_lowering=False, debug=debug,
                   num_devices=N_CORES)

    f32 = dt.float32
    bf = dt.bfloat16
    x_in = nc.dram_tensor("x", [N, D], bf, kind="ExternalInput").ap()
    id_in = nc.dram_tensor("ident", [128, 128], bf, kind="ExternalInput").ap()
    qt_in = nc.dram_tensor("qt", [D, J], bf, kind="ExternalInput").ap()
    w1_in = nc.dram_tensor("w1", [D, HC], bf, kind="ExternalInput").ap()
    b1_in = nc.dram_tensor("b1", [HC], f32, kind="ExternalInput").ap()
    w2_in = nc.dram_tensor("w2", [HC, D], bf, kind="ExternalInput").ap()
    b2_in = nc.dram_tensor("b2", [D], f32, kind="ExternalInput").ap()
    sc_in = nc.dram_tensor("sc", [2 + n_repeat], f32,
                           kind="ExternalInput").ap()  # [1/s0, 1/(2*s1), pad...]
    if general_path:
        g2_in = nc.dram_tensor("g2", [D], f32, kind="ExternalInput").ap()
        gb2_in = nc.dram_tensor("gb2", [D], f32, kind="ExternalInput").ap()
        bb_in = nc.dram_tensor("bb", [1], f32, kind="ExternalInput").ap()
        cj_in = nc.dram_tensor("cj", [J], f32, kind="ExternalInput").ap()
    out_ext = nc.dram_tensor("out", [N, D], bf, kind="ExternalOutput").ap()
    if taps:
        tp = {}
        for nm, shp in [("t_logits", [128, NT * J]), ("t_disp", [128, NT * J]),
                        ("t_comb", [128, NT * J]), ("t_ss", [128, NT]),
                        ("t_r", [128, NT]), ("t_slotsT", [J, D]),
                        ("t_recvT", [J, D]), ("t_h", [128, HT * J]),
                        ("t_oe", [J, D]), ("t_oall", [J, D]),
                        ("t_tau", [128, NT])]:
            tp[nm] = nc.dram_tensor(nm, shp, f32, kind="ExternalOutput").ap()

    a2a1_in = nc.dram_tensor("a2a1_in", [J, D], bf)
    a2a1_out = nc.dram_tensor("a2a1_out", [J, D], bf)
    a2a2_in = nc.dram_tensor("a2a2_in", [J, D], bf)
    a2a2_out = nc.dram_tensor("a2a2_out", [J, D], bf)
    groups = [list(range(N_CORES))]

    xv = x_in.rearrange("(t p) d -> t p d", p=128)
    ov = out_ext.rearrange("(t p) d -> t p d", p=128)

    with tile.TileContext(nc) as tc:
        with (
            tc.tile_pool(name="const", bufs=1) as constp,
            tc.tile_pool(name="xpool", bufs=14) as xpool,
            tc.tile_pool(name="xtp", bufs=5) as xtp,
            tc.tile_pool(name="batch", bufs=1) as batchp,
            tc.tile_pool(name="small", bufs=2) as smallp,
            tc.tile_pool(name="mlp", bufs=1) as mlpp,
            tc.tile_pool(name="s16", bufs=1) as s16p,
            tc.tile_pool(name="fin", bufs=4) as finp,
        ):
            # ---- constants (identity FIRST: phase A's transposes need it,
            # and it must not queue behind the weight DMAs -- loaded from
            # DRAM so no gpsimd affine_select lands on the Pool queue) ----
            identB = constp.tile([128, 128], bf)
            nc.sync.dma_start(out=identB[:], in_=id_in)

            # ---- weight prefetch (gpsimd/SWDGE; Pool is otherwise idle
            # until the first collective, so the transfer-time charge lands
            # on an idle engine and overlaps phase A) ----
            # w1sb[p, (dcc, h)] <- w1[dcc*128+p, h]; 8 MB in one DMA
            w1sb = constp.tile([128, DC * HC], bf)
            nc.gpsimd.dma_start(
                out=w1sb[:],
                in_=bass.AP(tensor=w1_in.tensor, offset=0,
                            ap=[[HC, 128], [128 * HC, DC], [1, HC]]))
            # w2sb[p, (ht, d)] <- w2[ht*128+p, d]; 8 MB in one DMA
            w2sb = constp.tile([128, HT * D], bf)
            nc.gpsimd.dma_start(
                out=w2sb[:],
                in_=bass.AP(tensor=w2_in.tensor, offset=0,
                            ap=[[D, 128], [128 * D, HT], [1, D]]))
            qt_sb = constp.tile([128, DC * J], bf)  # [d_local, (dc, j)]
            nc.sync.dma_start(
                out=qt_sb[:],
                in_=bass.AP(tensor=qt_in.tensor, offset=0,
                            ap=[[J, 128], [128 * J, DC], [1, J]]))
            inv_s0 = constp.tile([128, 1], f32)
            inv_2s1 = constp.tile([128, 1], f32)
            nc.sync.dma_start(out=inv_s0[:], in_=bass.AP(
                tensor=sc_in.tensor, offset=0, ap=[[0, 128], [1, 1]]))
            nc.sync.dma_start(out=inv_2s1[:], in_=bass.AP(
                tensor=sc_in.tensor, offset=1, ap=[[0, 128], [1, 1]]))
            if general_path:
                g2_sb = constp.tile([128, D], f32)
                nc.sync.dma_start(out=g2_sb[:], in_=bass.AP(
                    tensor=g2_in.tensor, offset=0, ap=[[0, 128], [1, D]]))
                gb2_sb = constp.tile([128, D], f32)
                nc.sync.dma_start(out=gb2_sb[:], in_=bass.AP(
                    tensor=gb2_in.tensor, offset=0, ap=[[0, 128], [1, D]]))
                bb_sb = constp.tile([128, 1], f32)
                nc.sync.dma_start(out=bb_sb[:], in_=bass.AP(
                    tensor=bb_in.tensor, offset=0, ap=[[0, 128], [1, 1]]))
                cj_sb = constp.tile([128, J], f32)
                nc.sync.dma_start(out=cj_sb[:], in_=bass.AP(
                    tensor=cj_in.tensor, offset=0, ap=[[0, 128], [1, J]]))
            b1_sb = constp.tile([128, HT], f32)  # [h_local, ht]
            nc.sync.dma_start(out=b1_sb[:], in_=bass.AP(
                tensor=b1_in.tensor, offset=0, ap=[[1, 128], [128, HT]]))
            b2_sb = constp.tile([J, D], bf)
            nc.gpsimd.dma_start(out=b2_sb[:], in_=bass.AP(
                tensor=b2_in.tensor, offset=0, ap=[[0, J], [1, D]]))

            for rep in range(n_repeat):
                # ======== PHASE A ========
                dispatch_all = batchp.tile([128, NT * J], bf, tag="da")
                ss_all = batchp.tile([128, NT], f32, tag="ss")
                r_all = batchp.tile([128, NT], f32, tag="rr")
                scratch = batchp.tile([128, D], f32, tag="scr")
                zb = batchp.tile([128, NT * J], f32, tag="zb")

                with (
                    tc.tile_pool(name="psA_tr", bufs=3, space="PSUM") as psA_tr,
                    tc.tile_pool(name="psA_la", bufs=1, space="PSUM") as psA_la,
                    tc.tile_pool(name="psA_slot", bufs=1, space="PSUM") as psA_slot,
                ):
                    # raw logits for the whole batch accumulate into ONE
                    # PSUM bank; softmax/entmax read it in place (no per-
                    # tile PSUM->SBUF copy).
                    logall_ps = psA_la.tile([128, NT * J], f32, tag="lap")
                    slotsT_ps = psA_slot.tile([J, D], f32, tag="slps")
                    x_tiles = []

                    def emit_supertile(st_idx):
                        # rsqrt + dispatch softmax + slots for one super-tile.
                        # Emitted half a super-tile late (software pipelining)
                        # so the in-order DVE queue never stalls tile ss ops
                        # behind a softmax that waits on PE/ACT logits.
                        i0 = st_idx * ST
                        ssv = ss_all[:, i0:i0 + ST]
                        rv = r_all[:, i0:i0 + ST]
                        bits = smallp.tile([128, ST], dt.int32, tag="bits")
                        nc.vector.tensor_scalar(
                            out=bits[:], in0=ssv.bitcast(dt.int32), scalar1=1,
                            scalar2=None, op0=ALU.arith_shift_right)
                        nc.vector.tensor_scalar(
                            out=bits[:], in0=bits[:], scalar1=-1,
                            scalar2=RSQRT_MAGIC, op0=ALU.mult, op1=ALU.add)
                        rf = bits[:].bitcast(f32)
                        half_ss = smallp.tile([128, ST], f32, tag="hss")
                        nc.vector.tensor_scalar_mul(half_ss[:], ssv, 0.5)
                        tmp = smallp.tile([128, ST], f32, tag="nrt")
                        # magic-constant seed has ~3.4% rel err; two Newton
                        # steps bring it to ~4e-6 (each squares the error).
                        for _ in range(2):
                            nc.vector.tensor_mul(tmp[:], rf, rf)
                            nc.vector.tensor_mul(tmp[:], tmp[:], half_ss[:])
                            nc.vector.tensor_scalar(
                                out=tmp[:], in0=tmp[:], scalar1=-1.0,
                                scalar2=1.5, op0=ALU.mult, op1=ALU.add)
                            nc.vector.tensor_mul(rf, rf, tmp[:])
                        nc.vector.tensor_copy(rv, rf)

                        r0 = smallp.tile([128, ST], f32, tag="r0")
                        nc.vector.tensor_scalar_mul(r0[:], rv, inv_s0[:])
                        lview = logall_ps[:, i0 * J:(i0 + ST) * J]
                        z0 = smallp.tile([128, ST * J], f32, tag="z0")
                        if general_path:
                            nc.vector.tensor_tensor(
                                out=z0[:].rearrange("p (i j) -> p i j", j=J),
                                in0=lview.rearrange("p (i j) -> p i j", j=J),
                                in1=bass.AP(tensor=cj_sb.tensor,
                                            offset=cj_sb[:].offset,
                                            ap=[cj_sb[:].ap[0], [0, ST],
                                                [1, J]]),
                                op=ALU.add)
                            nc.vector.tensor_tensor(
                                out=z0[:].rearrange("p (i j) -> p i j", j=J),
                                in0=z0[:].rearrange("p (i j) -> p i j", j=J),
                                in1=bass.AP(tensor=r0.tensor,
                                            offset=r0[:].offset,
                                            ap=[r0[:].ap[0], [1, ST], [0, J]]),
                                op=ALU.mult)
                        else:
                            nc.vector.tensor_tensor(
                                out=z0[:].rearrange("p (i j) -> p i j", j=J),
                                in0=lview.rearrange("p (i j) -> p i j", j=J),
                                in1=bass.AP(tensor=r0.tensor,
                                            offset=r0[:].offset,
                                            ap=[r0[:].ap[0], [1, ST], [0, J]]),
                                op=ALU.mult)
                        z0_ise = bass.AP(
                            tensor=z0.tensor, offset=z0[:].offset,
                            ap=[z0[:].ap[0], [J, ST], [1, S], [2, NCEXP]])
                        mx = smallp.tile([128, ST * S], f32, tag="mx")
                        nc.vector.tensor_reduce(
                            mx[:].rearrange("p (i s) -> p i s", s=S), z0_ise,
                            axis=AX.X, op=ALU.max)
                        mx_b = bass.AP(
                            tensor=mx.tensor, offset=mx[:].offset,
                            ap=[mx[:].ap[0], [S, ST], [1, S], [0, NCEXP]])
                        nc.vector.tensor_tensor(out=z0_ise, in0=z0_ise,
                                                in1=mx_b, op=ALU.subtract)
                        nc.scalar.activation(z0[:], z0[:], AF.Exp)
                        se = smallp.tile([128, ST * S], f32, tag="se")
                        nc.vector.tensor_reduce(
                            se[:].rearrange("p (i s) -> p i s", s=S), z0_ise,
                            axis=AX.X, op=ALU.add)
                        nc.vector.reciprocal(se[:], se[:])
                        se_b = bass.AP(
                            tensor=se.tensor, offset=se[:].offset,
                            ap=[se[:].ap[0], [S, ST], [1, S], [0, NCEXP]])
                        dview = dispatch_all[:, i0 * J:(i0 + ST) * J]
                        nc.vector.tensor_tensor(
                            out=bass.AP(
                                tensor=dview.tensor, offset=dview.offset,
                                ap=[dview.ap[0], [J, ST], [1, S], [2, NCEXP]]),
                            in0=z0_ise, in1=se_b, op=ALU.mult)

                        # slots accumulation: slotsT += dispatch_i.T @ x_i
                        for ii2 in range(ST):
                            i2 = i0 + ii2
                            for half in range(2):
                                nc.tensor.matmul(
                                    slotsT_ps[:, half * 512:(half + 1) * 512],
                                    dispatch_all[:, i2 * J:(i2 + 1) * J],
                                    x_tiles[i2][:, half * 512:(half + 1) * 512],
                                    start=(i2 == 0), stop=(i2 == NT - 1))

                    for st in range(NST):
                        for ii in range(ST):
                            i = st * ST + ii
                            xt = xpool.tile([128, D], bf, tag="xt")
                            nc.sync.dma_start(out=xt[:], in_=xv[i])
                            x_tiles.append(xt)
                            if not general_path:
                                # ss = sum(x^2): (x*1)*x with running accum.
                                # All non-scalar operands bf16 => DVE 2x mode.
                                nc.vector.scalar_tensor_tensor(
                                    out=scratch[:], in0=xt[:], scalar=1.0,
                                    in1=xt[:], op0=ALU.mult, op1=ALU.mult,
                                    accum_out=ss_all[:, i:i + 1])
                            else:
                                xf = smallp.tile([128, D], f32, tag="gs0")
                                nc.vector.tensor_copy(xf[:], xt[:])
                                t1 = smallp.tile([128, D], f32, tag="gs1")
                                nc.vector.tensor_mul(t1[:], xf[:], g2_sb[:])
                                nc.vector.scalar_tensor_tensor(
                                    out=t1[:], in0=t1[:], scalar=1.0,
                                    in1=xf[:], op0=ALU.mult, op1=ALU.mult,
                                    accum_out=ss_all[:, i:i + 1])
                                ss2 = smallp.tile([128, 1], f32, tag="gs3")
                                nc.vector.scalar_tensor_tensor(
                                    out=t1[:], in0=xf[:], scalar=1.0,
                                    in1=gb2_sb[:], op0=ALU.mult, op1=ALU.mult,
                                    accum_out=ss2[:])
                                nc.vector.tensor_add(
                                    ss_all[:, i:i + 1], ss_all[:, i:i + 1], ss2[:])
                                nc.vector.tensor_add(
                                    ss_all[:, i:i + 1], ss_all[:, i:i + 1], bb_sb[:])

                            # transpose x tile (8 chunks, bf16) -> xT
                            xT = xtp.tile([128, D], bf, tag="xT")
                            for half in range(2):
                                ptr = psA_tr.tile([128, 512], bf, tag="ptr")
                                for k in range(4):
                                    dcc = half * 4 + k
                                    nc.tensor.transpose(
                                        ptr[:, k * 128:(k + 1) * 128],
                                        xt[:, dcc * 128:(dcc + 1) * 128],
                                        identB[:])
                                nc.scalar.copy(
                                    xT[:, half * 512:(half + 1) * 512], ptr[:])

                            # logits_i = xT.T @ qT -> region i of the logits
                            # PSUM bank (accumulate over d-chunks)
                            for dcc in range(DC):
                                nc.tensor.matmul(
                                    logall_ps[:, i * J:(i + 1) * J],
                                    xT[:, dcc * 128:(dcc + 1) * 128],
                                    qt_sb[:, dcc * J:(dcc + 1) * J],
                                    start=(dcc == 0), stop=(dcc == DC - 1))

                            if ii == ST // 2 and st > 0:
                                emit_supertile(st - 1)
                    emit_supertile(NST - 1)

                    slotsT = s16p.tile([J, D], bf, tag="slt")
                    nc.vector.tensor_copy(slotsT[:], slotsT_ps[:])
                    nc.gpsimd.dma_start(out=a2a1_in[:], in_=slotsT[:])
                    if taps and rep == 0:
                        nc.sync.dma_start(out=tp["t_slotsT"], in_=slotsT[:])

                    # ---- entmax z2 built from the PSUM logits before the
                    # bank is released (rest of entmax runs during a2a1/B)
                    combine_all = batchp.tile([128, NT * J], bf, tag="ca")
                    r1 = smallp.tile([128, NT], f32, tag="r1")
                    nc.vector.tensor_scalar_mul(r1[:], r_all[:], inv_2s1[:])
                    z2v = zb[:, 0:NT * J]
                    if general_path:
                        nc.vector.tensor_tensor(
                            out=z2v.rearrange("p (i j) -> p i j", j=J),
                            in0=logall_ps[:].rearrange("p (i j) -> p i j", j=J),
                            in1=bass.AP(tensor=cj_sb.tensor,
                                        offset=cj_sb[:].offset,
                                        ap=[cj_sb[:].ap[0], [0, NT], [1, J]]),
                            op=ALU.add)
                        nc.vector.tensor_tensor(
                            out=z2v.rearrange("p (i j) -> p i j", j=J),
                            in0=z2v.rearrange("p (i j) -> p i j", j=J),
                            in1=bass.AP(tensor=r1.tensor, offset=r1[:].offset,
                                        ap=[r1[:].ap[0], [1, NT], [0, J]]),
                            op=ALU.mult)
                    else:
                        nc.vector.tensor_tensor(
                            out=z2v.rearrange("p (i j) -> p i j", j=J),
                            in0=logall_ps[:].rearrange("p (i j) -> p i j", j=J),
                            in1=bass.AP(tensor=r1.tensor, offset=r1[:].offset,
                                        ap=[r1[:].ap[0], [1, NT], [0, J]]),
                            op=ALU.mult)

                # ======== entmax combine weights (overlaps phase B) ========
                m16 = smallp.tile([128, NT], f32, tag="m16")
                nc.vector.tensor_reduce(
                    m16[:], z2v.rearrange("p (i j) -> p i j", j=J),
                    axis=AX.X, op=ALU.max)
                m16_b = bass.AP(tensor=m16.tensor, offset=m16[:].offset,
                                ap=[m16[:].ap[0], [1, NT], [0, J]])
                nc.vector.tensor_tensor(
                    out=z2v.rearrange("p (i j) -> p i j", j=J),
                    in0=z2v.rearrange("p (i j) -> p i j", j=J),
                    in1=m16_b, op=ALU.subtract)
                tau = smallp.tile([128, NT], f32, tag="tau")
                nc.vector.memset(tau[:], -1.0)
                # scratch (the ss dummy output) is dead after phase A;
                # reuse it for the Newton u / u^2 buffers (saves 4KB/part)
                ubuf = scratch[:, 0:NT * J]
                sqbuf = scratch[:, NT * J:2 * NT * J]

                s1t = smallp.tile([128, NT], f32, tag="s1t")
                s2t = smallp.tile([128, NT], f32, tag="s2t")
                for it in range(NEWTON_ITERS):
                    tau_b = bass.AP(tensor=tau.tensor, offset=tau[:].offset,
                                    ap=[tau[:].ap[0], [1, NT], [0, J]])
                    nc.vector.tensor_tensor(
                        out=ubuf.rearrange("p (i j) -> p i j", j=J),
                        in0=z2v.rearrange("p (i j) -> p i j", j=J),
                        in1=tau_b, op=ALU.subtract)
                    nc.vector.tensor_scalar_max(ubuf, ubuf, 0.0)
                    nc.vector.tensor_reduce(
                        s1t[:], ubuf.rearrange("p (i j) -> p i j", j=J),
                        axis=AX.X, op=ALU.add)
                    nc.vector.tensor_mul(sqbuf, ubuf, ubuf)
                    nc.vector.tensor_reduce(
                        s2t[:], sqbuf.rearrange("p (i j) -> p i j", j=J),
                        axis=AX.X, op=ALU.add)
                    nc.vector.tensor_scalar(
                        out=s2t[:], in0=s2t[:], scalar1=-1.0, scalar2=None,
                        op0=ALU.add)
                    nc.vector.tensor_scalar_mul(s1t[:], s1t[:], 2.0)
                    nc.vector.reciprocal(s1t[:], s1t[:])
                    nc.vector.tensor_mul(s1t[:], s1t[:], s2t[:])
                    nc.vector.tensor_add(tau[:], tau[:], s1t[:])
                tau_b = bass.AP(tensor=tau.tensor, offset=tau[:].offset,
                                ap=[tau[:].ap[0], [1, NT], [0, J]])
                nc.vector.tensor_tensor(
                    out=ubuf.rearrange("p (i j) -> p i j", j=J),
                    in0=z2v.rearrange("p (i j) -> p i j", j=J),
                    in1=tau_b, op=ALU.subtract)
                nc.vector.tensor_scalar_max(ubuf, ubuf, 0.0)
                nc.vector.tensor_mul(combine_all[:], ubuf, ubuf)
                if taps and rep == 0:
                    nc.sync.dma_start(out=tp["t_logits"], in_=zb[:])
                    nc.sync.dma_start(out=tp["t_disp"], in_=dispatch_all[:])
                    nc.sync.dma_start(out=tp["t_comb"], in_=combine_all[:])
                    nc.sync.dma_start(out=tp["t_ss"], in_=ss_all[:])
                    nc.sync.dma_start(out=tp["t_r"], in_=r_all[:])
                    nc.sync.dma_start(out=tp["t_tau"], in_=tau[:])

                with tc.tile_pool(name="psC_tr", bufs=2,
                                  space="PSUM") as psC_tr:
                    combT = mlpp.tile([J, NT * 128], bf, tag="cT")
                    for i in range(NT):
                        ptr = psC_tr.tile([J, 128], bf, tag="ptr")
                        nc.tensor.transpose(
                            ptr[:], combine_all[:, i * J:(i + 1) * J], identB[:])
                        nc.scalar.copy(combT[:, i * 128:(i + 1) * 128], ptr[:])
                if stop_after < 1:
                    continue
                nc.gpsimd.collective_compute(
                    "AllToAll", ALU.bypass, replica_groups=groups,
                    ins=[a2a1_in[:].opt()], outs=[a2a1_out[:].opt()])
                recvT = s16p.tile([J, D], bf, tag="rcv")
                nc.gpsimd.dma_start(out=recvT[:], in_=a2a1_out[:])
                if taps and rep == 0:
                    nc.sync.dma_start(out=tp["t_recvT"], in_=recvT[:])

                # ======== PHASE B: expert MLP (weights pre-resident) ========
                if stop_after < 2:
                    continue
                with (
                    tc.tile_pool(name="psB_tr", bufs=2, space="PSUM") as psB_tr,
                    tc.tile_pool(name="psB_h", bufs=1, space="PSUM") as psB_h,
                    tc.tile_pool(name="psB_o", bufs=1, space="PSUM") as psB_o,
                ):
                    sT = mlpp.tile([128, DC * J], bf, tag="sT")
                    for dcc in range(DC):
                        ptr = psB_tr.tile([128, J], bf, tag="ptr")
                        nc.tensor.transpose(
                            ptr[:], recvT[:, dcc * 128:(dcc + 1) * 128],
                            identB[0:J, 0:J])
                        nc.scalar.copy(sT[:, dcc * J:(dcc + 1) * J], ptr[:])

                    h_ps = psB_h.tile([128, HT * J], f32, tag="hps")
                    for dcc in range(DC):
                        for ht in range(HT):
                            # single accumulation group for the whole bank:
                            # start=True clears has_written bank-wide, so
                            # only the very first matmul starts.
                            nc.tensor.matmul(
                                h_ps[:, ht * J:(ht + 1) * J],
                                w1sb[:, dcc * HC + ht * 128:
                                     dcc * HC + (ht + 1) * 128],
                                sT[:, dcc * J:(dcc + 1) * J],
                                start=(dcc == 0 and ht == 0),
                                stop=(dcc == DC - 1 and ht == HT - 1))
                    h_sb = mlpp.tile([128, HT * J], f32, tag="hsb")
                    nc.vector.tensor_tensor(
                        out=h_sb[:].rearrange("p (t j) -> p t j", j=J),
                        in0=h_ps[:].rearrange("p (t j) -> p t j", j=J),
                        in1=bass.AP(tensor=b1_sb.tensor, offset=b1_sb[:].offset,
                                    ap=[b1_sb[:].ap[0], [1, HT], [0, J]]),
                        op=ALU.add)
                    h_sbB = mlpp.tile([128, HT * J], bf, tag="hsbB")
                    # sim_safe: CoreSim lacks Gelu; Sigmoid has identical
                    # engine timing (both ACT table lookups).
                    nc.scalar.activation(h_sbB[:], h_sb[:],
                                         AF.Sigmoid if sim_safe else AF.Gelu)
                    if taps and rep == 0:
                        nc.sync.dma_start(out=tp["t_h"], in_=h_sb[:])

                    o_ps = psB_o.tile([J, D], f32, tag="ops")
                    for ht in range(HT):
                        for half in range(2):
                            nc.tensor.matmul(
                                o_ps[:, half * 512:(half + 1) * 512],
                                h_sbB[:, ht * J:(ht + 1) * J],
                                w2sb[:, ht * D + half * 512:
                                     ht * D + (half + 1) * 512],
                                start=(ht == 0), stop=(ht == HT - 1))
                    oe_sb = s16p.tile([J, D], bf, tag="oe")
                    nc.vector.tensor_add(oe_sb[:], o_ps[:], b2_sb[:])
                    nc.gpsimd.dma_start(out=a2a2_in[:], in_=oe_sb[:])
                    if taps and rep == 0:
                        nc.sync.dma_start(out=tp["t_oe"], in_=oe_sb[:])

                if stop_after < 3:
                    continue
                nc.gpsimd.collective_compute(
                    "AllToAll", ALU.bypass, replica_groups=groups,
                    ins=[a2a2_in[:].opt()], outs=[a2a2_out[:].opt()])
                out_all = s16p.tile([J, D], bf, tag="oall")
                nc.gpsimd.dma_start(out=out_all[:], in_=a2a2_out[:])
                if taps and rep == 0:
                    nc.sync.dma_start(out=tp["t_oall"], in_=out_all[:])

                # ======== PHASE C: final combine matmul ========
                if stop_after < 4:
                    continue
                with (
                    tc.tile_pool(name="psC_fin", bufs=4, space="PSUM") as psC_fin,
                ):
                    for i in range(NT):
                        fps = psC_fin.tile([128, D], f32, tag="fps")
                        for half in range(2):
                            nc.tensor.matmul(
                                fps[:, half * 512:(half + 1) * 512],
                                combT[:, i * 128:(i + 1) * 128],
                                out_all[:, half * 512:(half + 1) * 512],
                                start=True, stop=True)
                        fsb = finp.tile([128, D], bf, tag="fsb")
                        if i % 2 == 0:
                            nc.vector.tensor_copy(fsb[:], fps[:])
                            nc.sync.dma_start(out=ov[i], in_=fsb[:])
                        else:
                            nc.scalar.copy(fsb[:], fps[:])
                            nc.gpsimd.dma_start(out=ov[i], in_=fsb[:])

    nc.compile()
    return nc


def _host_prep(inputs):
    """Host-side tiny prep: normalized core-expert queries (e-major rows)."""
    phi = np.asarray(inputs["phi"], np.float32)[:NCEXP]        # [8, 2, D]
    qg = np.asarray(inputs["query_gamma"], np.float32)
    qb = np.asarray(inputs["query_beta"], np.float32)
    lg = np.asarray(inputs["ln_gamma"], np.float32)
    lb = np.asarray(inputs["ln_beta"], np.float32)
    q = phi * qg + qb
    mu = q.mean(-1, keepdims=True, dtype=np.float32)
    var = ((q - mu) ** 2).mean(-1, keepdims=True, dtype=np.float32)
    q = ((q - mu) / np.sqrt(var + 1e-5)).astype(np.float32) * lg + lb
    q = q / (np.sqrt((q * q).sum(-1, keepdims=True, dtype=np.float32)) + L2_EPS)
    q = q.astype(np.float32).reshape(J, D)                     # rows j = 2e + s
    kg = np.asarray(inputs["key_gamma"], np.float32)
    kb = np.asarray(inputs["key_beta"], np.float32)
    general = not (np.all(kg == 1.0) and np.all(kb == 0.0))
    s0 = float(np.asarray(inputs["scale0"], np.float32))
    s1 = float(np.asarray(inputs["scale1"], np.float32))
    sc = np.array([1.0 / s0, 1.0 / (2.0 * s1)], np.float32)
    prep = {"q": q, "sc": sc, "general": general}
    if general:
        prep["qt"] = np.ascontiguousarray((q * kg[None, :]).T).astype(BF16)
        prep["g2"] = (kg * kg).astype(np.float32)
        prep["gb2"] = (2.0 * kg * kb).astype(np.float32)
        prep["bb"] = np.array([float((kb * kb).sum())], np.float32)
        prep["cj"] = (q @ kb).astype(np.float32)
    else:
        prep["qt"] = np.ascontiguousarray(q.T).astype(BF16)
    return prep


def make_in_maps(inputs, prep, n_repeat=1):
    x = np.asarray(inputs["x"], np.float32)
    cw1 = np.asarray(inputs["core_w1"], np.float32)
    cb1 = np.asarray(inputs["core_b1"], np.float32)
    cw2 = np.asarray(inputs["core_w2"], np.float32)
    cb2 = np.asarray(inputs["core_b2"], np.float32)
    ident = np.eye(128, dtype=np.float32).astype(BF16)
    in_maps = []
    for c in range(N_CORES):
        m = {
            "x": np.ascontiguousarray(x[c]).astype(BF16),
            "ident": ident,
            "qt": prep["qt"],
            "w1": np.ascontiguousarray(cw1[c]).astype(BF16),
            "b1": np.ascontiguousarray(cb1[c]),
            "w2": np.ascontiguousarray(cw2[c]).astype(BF16),
            "b2": np.ascontiguousarray(cb2[c]),
            "sc": np.concatenate([prep["sc"], np.zeros(n_repeat, np.float32)]),
        }
        if prep["general"]:
            m["g2"] = prep["g2"]
            m["gb2"] = prep["gb2"]
            m["bb"] = prep["bb"]
            m["cj"] = prep["cj"]
        in_maps.append(m)
    return in_maps


def kernel(**inputs) -> np.ndarray:
    prep = _host_prep(inputs)
    nc = build_nc(n_repeat=1, general_path=prep["general"])
    in_maps = make_in_maps(inputs, prep)
    res = run_bass_kernel_spmd(nc, in_maps, core_ids=list(range(N_CORES)))
    out = np.stack([res.results[c]["out"] for c in range(N_CORES)], axis=0)
    return out.astype(np.float32)

